# revision 21
# baseline (speedup 1.0000x reference)
"""Trainium2 Bass kernel: multi-head relational module (dense_transformer).

Computation (per batch b):
  xin = concat(x, xy-coords)                 [N=1024, FEAT=26]
  Q/K/V = LN_global(xin @ Wp.T + bp)         LN over all (heads, N, D) per b
  S1 = elu(Q @ qlw.T + qlb + K @ klw.T + klb)      [h, N, N]
  A  = softmax(S1 @ alw.T + alb, axis=-1)          [h, N, N]
  E  = relu((A @ V).reshape(N, 256) @ l1w.T + l1b) [N, 64]
  out[b] = max_n LN(E)                             [64]

Sharding: data-parallel over batch (B=16 -> 2 per core on 8 cores).

Precision plan:
  * Projections, S1 additive-attention matmuls: fp16/bf16 (fp32 PSUM accum).
  * The two big contractions (S1 @ alw.T at 84% of FLOPs, and A @ V) run in
    fp8e4 (e4m3) with perf_mode=DoubleRow: contraction chunks are paired
    [128, 2, free] so one matmul instruction contracts 256 rows (~2x
    TensorE throughput, measured 216 ns per 128x256x512 matmul). alw is
    pre-scaled by ALW_SCALE on the host to move its mass out of the e4m3
    subnormal range; the softmax exp un-scales via its `scale` input.

Engine balance choices (from perfetto/ntff analysis):
  * S1 elu chain per 128x1024 chunk is ONE 2-bank PSUM tile + 3 ops:
      ep = exp(pss + posb)              [ACT]
      qq = min(ep, 1)                   [DVE 4x tensor_scalar, immediates]
      st = (pss max -posb) + qq         [DVE, the only PSUM read]
    st == elu(u)+1-posb; the -posb and the "+1" are folded into albe on
    the host (albe = alb - sum_m alw_q[:,m] * (1 - posb[m])).
  * All LayerNorm partition-reductions and the softmax-denominator
    broadcast run on the otherwise-idle GpSimd engine
    (partition_all_reduce / partition_broadcast), which frees the PE from
    ~60 small matmuls and frees a PSUM bank (no pmisc pool).
  * LayerNorm rstd is computed as exp(-0.5*ln(v+eps)) so the ACT engine
    stays in the single `natural_log_exp_and_others` table set.
"""

import os
from contextlib import ExitStack

import ml_dtypes
import numpy as np

import concourse.bass as bass
import concourse.bass_utils as bass_utils
import concourse.mybir as mybir
import concourse.tile as tile
from concourse import bacc
from concourse.bass_isa import ReduceOp

AF = mybir.ActivationFunctionType
ALU = mybir.AluOpType
FP32 = mybir.dt.float32
F8 = mybir.dt.float8e4
DR = mybir.MatmulPerfMode.DoubleRow

B, F, IH, IW = 16, 24, 32, 32
N = IH * IW          # 1024
HEADS, D = 4, 64
HD = HEADS * D       # 256
FEAT = F + 2         # 26
F1 = FEAT + 1        # 27 (ones row folds the projection bias in)
EPS = 1e-5
NCORES = 8
BL = B // NCORES     # batches per core
NT = N // 128        # 8 chunks of the node axis
NJ = NT // 2         # 4 DoubleRow chunk-pairs
NPAIRS = BL * HEADS  # 8 (b, h) pairs per core
ALW_SCALE = 16.0     # host-side pre-scale of alw before e4m3 quantization


def _build(mmdt, debug=False):
    """Build + compile the per-core Bass program. Same program on all cores."""
    nc = bacc.Bacc("TRN2", target_bir_lowering=False, debug=False)

    def din(name, shape, dt=FP32):
        return nc.dram_tensor(name, shape, dt, kind="ExternalInput").ap()

    xt_d = din("xt", [BL, F1, N], mmdt)
    wqk_d = din("wqk", [F1, 512], mmdt)
    wv_d = din("wv", [F1, HD], mmdt)
    wcat_d = din("wcat", [128, N], mmdt)
    alwt_d = din("alwt", [NT, 128, N], F8)
    l1wt_d = din("l1wt", [2, 128, D], mmdt)
    smalls_d = din("smalls", [128, 3 * NT + 3])
    out_d = nc.dram_tensor("out", [BL, D], FP32, kind="ExternalOutput").ap()

    dbg = {}
    if debug:
        for nm, shape, dt in [
            ("dbg_qkcat", [NPAIRS, 128, N], mmdt),
            ("dbg_vt", [BL * NJ, 128, 2, HEADS, 128], F8),
            ("dbg_s1t", [NJ, 128, 2, N], F8),
            ("dbg_expt", [NJ, 128, 2, N], F8),
            ("dbg_eflat", [BL, 2, 128, N], mmdt),
            ("dbg_orelu", [BL, 64, N], FP32),
        ]:
            dbg[nm] = nc.dram_tensor(nm, shape, dt, kind="ExternalOutput").ap()

    with tile.TileContext(nc) as tc, ExitStack() as ctx:
        pool = lambda name, bufs, **kw: ctx.enter_context(
            tc.tile_pool(name=name, bufs=bufs, **kw)
        )
        # PSUM: 8 banks total, budgeted exactly:
        #   pa2: 2 bufs x 1 bank (A2 accum; also QK/V proj + out-lin psums)
        #   ps1: 2 bufs x 2 banks (merged-half S1 psum [128,1024])
        #   pe:  2 bufs x 1 bank (E accumulation halves)
        pa2 = pool("pa2", 2, space="PSUM")
        ps1 = pool("ps1", 2, space="PSUM")
        pe_ = pool("pe", 2, space="PSUM")

        consts = pool("consts", 1)
        sxt = pool("sxt", 2)
        sqkraw = pool("sqkraw", 9)
        svraw = pool("svraw", 17)
        sqkcat = pool("sqkcat", NPAIRS)
        svt = pool("svt", BL * NJ)
        ss1t = pool("ss1t", 12)
        sexpt = pool("sexpt", 8)
        selu = pool("selu", 8)
        seflat = pool("seflat", 4)
        sorelu = pool("sorelu", 2)
        ssmall = pool("ssmall", 4)
        ssb = pool("ssb", 6)

        # ---- constants into SBUF (small/urgent first; big weights last) ----
        wqk_s = consts.tile([F1, 512], mmdt)
        nc.sync.dma_start(out=wqk_s, in_=wqk_d)
        wv_s = consts.tile([F1, HD], mmdt)
        nc.sync.dma_start(out=wv_s, in_=wv_d)
        xt_tiles = []
        for b in range(BL):
            xt_s = sxt.tile([F1, N], mmdt, name=f"xt{b}", tag="xt")
            nc.scalar.dma_start(out=xt_s, in_=xt_d[b])
            xt_tiles.append(xt_s)
        smalls_s = consts.tile([128, 3 * NT + 3], FP32)
        nc.sync.dma_start(out=smalls_s, in_=smalls_d)
        posb_s = smalls_s[:, 0:NT]
        nposb_s = smalls_s[:, NT:2 * NT]
        albe_s = smalls_s[:, 2 * NT:3 * NT]
        ind_s = smalls_s[:, 3 * NT:3 * NT + 2]
        l1b_s = smalls_s[0:D, 3 * NT + 2:3 * NT + 3]
        eps_s = consts.tile([128, 1], FP32)
        nc.vector.memset(eps_s, EPS)
        wcat_s = consts.tile([128, N], mmdt)
        nc.sync.dma_start(out=wcat_s, in_=wcat_d)
        alwt_s = consts.tile([128, NT, N], F8)
        for mi in range(NT):
            nc.sync.dma_start(out=alwt_s[:, mi, :], in_=alwt_d[mi])
        l1wt_s = consts.tile([128, 2, D], mmdt)
        for c in range(2):
            nc.sync.dma_start(out=l1wt_s[:, c, :], in_=l1wt_d[c])

        qkcat = {}   # (b, h) -> [128, N] mmdt   rows: 0-63 Q_h.T dims, 64-127 K_h.T
        vt = {}      # (b, j) -> [128, 2, HEADS, 80] F8 (col 64 per head = ones)
        eflat = {}   # (b, c) -> [128, N] mmdt   E.T stacked by head pairs

        def ln_scalars(stats, parts, cnt, name):
            """From SBUF stats [parts,3] = (sum mean_p, sum var_p, sum mean_p^2)
            build sbuf [parts,2] = (rstd, -mean*rstd). All per-partition."""
            stm = ssmall.tile([parts, 8], FP32, name=f"stm_{name}", tag="stm")
            nc.scalar.activation(stm[:, 0:3], stats, AF.Copy, scale=1.0 / cnt)
            nc.vector.tensor_mul(stm[:, 3:4], stm[:, 0:1], stm[:, 0:1])
            nc.vector.tensor_add(stm[:, 4:5], stm[:, 1:2], stm[:, 2:3])
            nc.vector.tensor_sub(stm[:, 5:6], stm[:, 4:5], stm[:, 3:4])
            # rstd = exp(-0.5 * ln(var + eps)); keeps ACT in one table set
            nc.scalar.activation(stm[:, 6:7], stm[:, 5:6], AF.Ln, bias=eps_s[0:parts])
            rhs2 = ssb.tile([parts, 2], FP32, name=f"rhs2_{name}", tag="sbb")
            nc.scalar.activation(rhs2[:, 0:1], stm[:, 6:7], AF.Exp, scale=-0.5)
            nc.vector.tensor_mul(stm[:, 7:8], stm[:, 0:1], rhs2[:, 0:1])
            nc.vector.tensor_scalar(
                rhs2[:, 1:2], stm[:, 7:8], -1.0, None, ALU.mult
            )
            return rhs2

        def agg_stats(st6, parts, name):
            """bn_aggr the [parts, nsub, 6] bn_stats + mean^2 -> [parts, 3]."""
            rhs3 = ssmall.tile([parts, 3], FP32, name=f"rhs3_{name}", tag="rhs3")
            nc.vector.bn_aggr(rhs3[:, 0:2], st6)
            nc.vector.tensor_mul(rhs3[:, 2:3], rhs3[:, 0:1], rhs3[:, 0:1])
            return rhs3

        # ================= stage 0: projections + LayerNorms =================
        def emit_stage0_qk(b):
            xt_s = xt_tiles[b]

            # --- combined Q|K projection, head-interleaved, transposed ---
            # batch 0 runs before the pair loop: borrow the idle ps1 pool so
            # four proj psums are in flight instead of two
            qkraw = []
            qst6 = ssmall.tile([128, 2 * HEADS, 6], FP32,
                               name=f"qst6_{b}", tag="st6")
            for h in range(HEADS):
                pools = {0: (pa2, "pa2"), 1: (ps1, "ps1"),
                         2: (pe_, "pe"), 3: (pa2, "pa2")}
                pp, tg = pools[h]
                ps = pp.tile([128, 512], FP32, name=f"pqk{b}{h}", tag=tg)
                ps2 = pp.tile([128, 512], FP32, name=f"pqk2{b}{h}", tag=tg)
                lhs = wqk_s[:, h * 128:(h + 1) * 128]
                nc.tensor.matmul(ps, lhs, xt_s[:, 0:512])
                nc.tensor.matmul(ps2, lhs, xt_s[:, 512:1024])
                raw = sqkraw.tile([128, N], FP32, name=f"qkraw{b}{h}", tag="qkraw")
                nc.scalar.activation(raw[:, 0:512], ps, AF.Copy)
                nc.scalar.activation(raw[:, 512:1024], ps2, AF.Copy)
                qkraw.append(raw)
                hv = raw.rearrange("p (s f) -> p s f", s=2)
                nc.vector.bn_stats(qst6[:, 2 * h, :], hv[:, 0, :])
                nc.vector.bn_stats(qst6[:, 2 * h + 1, :], hv[:, 1, :])
            rhs3 = agg_stats(qst6, 128, f"qk{b}")
            # rows 0-63 hold Q-dim stats, 64-127 K-dim stats: group-reduce via
            # masked columns + one full-partition all-reduce, then select.
            acc6 = ssmall.tile([128, 6], FP32, name=f"acc6_{b}", tag="acc6")
            nc.vector.memset(acc6, 0.0)
            nc.vector.tensor_copy(acc6[0:64, 0:3], rhs3[0:64, :])
            nc.vector.tensor_copy(acc6[64:128, 3:6], rhs3[64:128, :])
            nc.gpsimd.partition_all_reduce(acc6, acc6, 128, ReduceOp.add)
            sel = ssmall.tile([128, 3], FP32, name=f"sel_{b}", tag="rhs3")
            nc.vector.tensor_scalar(
                sel, acc6[:, 0:3], ind_s[:, 0:1], None, ALU.mult
            )
            nc.vector.scalar_tensor_tensor(
                sel, acc6[:, 3:6], ind_s[:, 1:2], sel, ALU.mult, ALU.add
            )
            sbb = ln_scalars(sel, 128, 64.0, f"qk{b}")
            for h in range(HEADS):
                qk = sqkcat.tile([128, N], mmdt, name=f"qkcat{b}{h}", tag="qkcat")
                nc.vector.tensor_scalar(
                    qk, qkraw[h], sbb[:, 0:1], sbb[:, 1:2], ALU.mult, ALU.add
                )
                qkcat[(b, h)] = qk

        def emit_stage0_v(b):
            xt_s = xt_tiles[b]
            # --- V projection, natural [n, hd] layout ---
            vraws = []
            vst6 = ssmall.tile([128, NT, 6], FP32, name=f"vst6_{b}", tag="st6")
            for nt in range(NT):
                pp = ps1 if nt % 2 == 1 else pa2
                tg = "ps1" if nt % 2 == 1 else "pa2"
                psv = pp.tile([128, HD], FP32, name=f"pv{b}{nt}", tag=tg)
                nc.tensor.matmul(
                    psv, xt_s[:, nt * 128:(nt + 1) * 128], wv_s
                )
                vr = svraw.tile([128, HD], FP32, name=f"vraw{b}{nt}", tag="vraw")
                nc.scalar.activation(vr, psv, AF.Copy)
                vraws.append(vr)
                nc.vector.bn_stats(vst6[:, nt, :], vr)
            rhs3v = agg_stats(vst6, 128, f"v{b}")
            nc.gpsimd.partition_all_reduce(rhs3v, rhs3v, 128, ReduceOp.add)
            sbv = ln_scalars(rhs3v, 128, 128.0, f"v{b}")
            for nt in range(NT):
                j, c = divmod(nt, 2)
                if c == 0:
                    v = svt.tile([128, 2, HEADS, 128], F8,
                                 name=f"vt{b}{j}", tag="vt")
                    vt[(b, j)] = v
                v = vt[(b, j)]
                nc.vector.tensor_scalar(
                    v[:, c, :, 64:128],
                    vraws[nt].rearrange("p (h d) -> p h d", h=HEADS),
                    sbv[:, 0:1], sbv[:, 1:2], ALU.mult, ALU.add,
                )
                nc.vector.memset(v[:, c, :, 0:1], 1.0)
                nc.vector.memset(v[:, c, :, 1:64], 0.0)

        # ================= stage 1: attention pipeline over (b, h) ==========
        s1p = {}    # (i, j) -> [128, 2, N] F8    S1.T chunk pair (elu+1-posb)
        exp2 = {}   # (i, j) -> [128, 2, N] F8    exp(A2T) chunk pair
        eps_ = {}   # (i, half) -> psum [65, 512] E.T accumulation + denominator

        def emit_s1_chunk(i, mi):
            b, h = divmod(i, HEADS)
            j, c = divmod(mi, 2)
            if (i, j) not in s1p:
                s1p[(i, j)] = ss1t.tile(
                    [128, 2, N], F8, name=f"s1p{i}_{j}", tag="s1t"
                )
            sp = s1p[(i, j)]
            pss = ps1.tile([128, N], FP32, name=f"ps1_{i}_{mi}", tag="ps1")
            wc = wcat_s[:, mi * 128:(mi + 1) * 128]
            nc.tensor.matmul(pss[:, 0:512], wc, qkcat[(b, h)][:, 0:512])
            nc.tensor.matmul(pss[:, 512:1024], wc, qkcat[(b, h)][:, 512:1024])
            # st = relu(u) + min(exp(u), 1) - posb, u = pss + posb:
            #   ep = exp(u)                 [ACT, PSUM->SBUF]
            #   qq = min(ep, 1)             [DVE 4x tensor_scalar, immediate]
            #   st = (pss max -posb) + qq   [DVE, the only PSUM read]
            ep = selu.tile([128, N], mmdt, name=f"ep{i}{mi}", tag="ep")
            nc.scalar.activation(ep, pss, AF.Exp, bias=posb_s[:, mi:mi + 1])
            qq = selu.tile([128, N], mmdt, name=f"qq{i}{mi}", tag="qq")
            nc.vector.tensor_scalar(qq, ep, 1.0, None, ALU.min)
            nc.vector.scalar_tensor_tensor(
                sp[:, c, :], pss, nposb_s[:, mi:mi + 1], qq, ALU.max, ALU.add
            )

        def emit_a2_ptile(i, pt):
            j2, c2 = divmod(pt, 2)
            if (i, j2) not in exp2:
                exp2[(i, j2)] = sexpt.tile(
                    [128, 2, N], F8, name=f"expt{i}_{j2}", tag="expt"
                )
            ex = exp2[(i, j2)]
            for half in range(2):
                sl = slice(half * 512, half * 512 + 512)
                psa = pa2.tile([128, 512], FP32,
                               name=f"pa2_{i}_{pt}{half}", tag="pa2")
                for j in range(NJ):
                    nc.tensor.matmul(
                        psa,
                        alwt_s[:, 2 * j:2 * j + 2, pt * 128:(pt + 1) * 128],
                        s1p[(i, j)][:, :, sl],
                        start=(j == 0), stop=(j == NJ - 1),
                        perf_mode=DR,
                    )
                nc.scalar.activation(
                    ex[:, c2, sl], psa, AF.Exp,
                    bias=albe_s[:, pt:pt + 1], scale=1.0 / ALW_SCALE,
                )

        def emit_e_mms(i, j):
            b, h = divmod(i, HEADS)
            for half in range(2):
                if j == 0:
                    eps_[(i, half)] = pe_.tile(
                        [128, 512], FP32, name=f"pe{i}{half}", tag="pe"
                    )
                sl = slice(half * 512, half * 512 + 512)
                nc.tensor.matmul(
                    eps_[(i, half)], vt[(b, j)][:, :, h, 0:128],
                    exp2[(i, j)][:, :, sl],
                    start=(j == 0), stop=(j == NJ - 1),
                    perf_mode=DR,
                )

        def emit_div(i):
            b, h = divmod(i, HEADS)
            c, po = h // 2, (h % 2) * 64
            if (b, c) not in eflat:
                eflat[(b, c)] = seflat.tile(
                    [128, N], mmdt, name=f"eflat{b}{c}", tag="eflat"
                )
            for half in range(2):
                sl = slice(half * 512, half * 512 + 512)
                pE = eps_[(i, half)]
                # ones column is FIRST in vt (V dims at 32:96 for the 32-aligned
                # partition-base rule), so the denominator is PSUM row 0
                # (a zero-base-partition read, which DVE handles; other
                # PSUM base partitions silently read partition 0 on HW).
                rc = ssmall.tile([1, 512], FP32, name=f"rc{i}{half}", tag="rc")
                nc.vector.reciprocal_approx_fast(out=rc, in_=pE[0:1, :])
                bc = ssb.tile([64, 512], FP32, name=f"bc{i}{half}", tag="bc")
                nc.gpsimd.partition_broadcast(bc, rc, channels=64)
                nc.vector.tensor_mul(
                    eflat[(b, c)][po:po + 64, sl], pE[64:128, :], bc
                )

        def emit_outlin(b):
            orl = sorelu.tile([64, N], FP32, name=f"orelu{b}", tag="orelu")
            ost6 = ssmall.tile([64, 2, 6], FP32, name=f"ost6_{b}", tag="ost6")
            rmax2 = ssmall.tile([64, 2], FP32, name=f"rmax2_{b}", tag="rmax2")
            for half in range(2):
                sl = slice(half * 512, half * 512 + 512)
                pso = pa2.tile([64, 512], FP32,
                               name=f"po{b}{half}", tag="pa2")
                for c in range(2):
                    nc.tensor.matmul(
                        pso, l1wt_s[:, c, :], eflat[(b, c)][:, sl],
                        start=(c == 0), stop=(c == 1),
                    )
                nc.scalar.activation(orl[:, sl], pso, AF.Relu, bias=l1b_s)
                nc.vector.bn_stats(ost6[:, half, :], orl[:, sl])
                nc.vector.reduce_max(rmax2[:, half:half + 1], orl[:, sl],
                                     axis=mybir.AxisListType.X)
            rhs3o = agg_stats(ost6, 64, f"o{b}")
            nc.gpsimd.partition_all_reduce(rhs3o, rhs3o, 64, ReduceOp.add)
            sbo = ln_scalars(rhs3o, 64, 64.0, f"o{b}")
            rmax = ssmall.tile([64, 1], FP32, name=f"rmax{b}", tag="rmax")
            nc.vector.tensor_max(rmax, rmax2[:, 0:1], rmax2[:, 1:2])
            ob = ssmall.tile([64, 1], FP32, name=f"ob{b}", tag="ob")
            nc.vector.tensor_scalar(
                ob, rmax, sbo[:, 0:1], sbo[:, 1:2], ALU.mult, ALU.add
            )
            nc.sync.dma_start(
                out=out_d[b:b + 1, :].rearrange("o d -> d o"), in_=ob
            )
            if debug:
                nc.sync.dma_start(out=dbg["dbg_orelu"][b], in_=orl)

        emit_stage0_qk(0)
        emit_stage0_v(0)
        emit_stage0_qk(1)
        emit_stage0_v(1)
        for mi in range(NT):
            emit_s1_chunk(0, mi)
        for i in range(NPAIRS):
            b, h = divmod(i, HEADS)
            for pt in range(NT):
                # prefetch next pair's S1 chunks one p-tile ahead so the elu
                # chain finishes before A2(i+1) starts consuming them
                if i + 1 < NPAIRS:
                    if pt == 0:
                        emit_s1_chunk(i + 1, 0)
                    if pt < NT - 1:
                        emit_s1_chunk(i + 1, pt + 1)
                if pt == 0 and i > 0:
                    # deferred: previous pair's softmax division (its recip
                    # chain latency hides behind A2(i, 0..4))
                    emit_div(i - 1)
                if pt == 5 and i > 0 and h == 0:
                    # five p-tiles later so the div chain (DVE recip -> gpsimd
                    # broadcast -> DVE mul) is done before the PE needs eflat
                    emit_outlin(b - 1)
                if pt >= 2 and pt % 2 == 0:
                    emit_e_mms(i, pt // 2 - 1)
                emit_a2_ptile(i, pt)
            emit_e_mms(i, NJ - 1)
            if debug and i == 0:
                for k in range(NJ):
                    nc.sync.dma_start(out=dbg["dbg_s1t"][k], in_=s1p[(0, k)])
                    nc.sync.dma_start(out=dbg["dbg_expt"][k], in_=exp2[(0, k)])
        emit_div(NPAIRS - 1)
        emit_outlin(BL - 1)

        if debug:
            for (b, h), t in qkcat.items():
                nc.sync.dma_start(out=dbg["dbg_qkcat"][b * HEADS + h], in_=t)
            for (b, j), t in vt.items():
                nc.sync.dma_start(out=dbg["dbg_vt"][b * NJ + j], in_=t)
            for (b, c), t in eflat.items():
                nc.sync.dma_start(out=dbg["dbg_eflat"][b, c], in_=t)

    import concourse.bacc as bacc_mod
    from concourse.hw_specs import get_activation_tables

    full = get_activation_tables(nc.m.arch)
    mine = full["natural_log_exp_and_others"]
    # Keep dict order/length (act_func_set_id indexes the full list); make
    # every other set unable to serve our functions so one set is loaded once.
    pinned = {
        name: (fns if name == "natural_log_exp_and_others" else fns - mine)
        for name, fns in full.items()
    }
    orig_gat = bacc_mod.get_activation_tables
    bacc_mod.get_activation_tables = lambda arch: pinned
    try:
        nc.compile()
    finally:
        bacc_mod.get_activation_tables = orig_gat
    return nc


def _prep_inputs(inputs, mmdt_np):
    """Host-side: shard + lay out all tensors exactly as SBUF wants them."""
    f = lambda a: np.ascontiguousarray(np.asarray(a, np.float32))
    x = f(inputs["x"])
    qpw, qpb = f(inputs["qpw"]), f(inputs["qpb"])
    kpw, kpb = f(inputs["kpw"]), f(inputs["kpb"])
    vpw, vpb = f(inputs["vpw"]), f(inputs["vpb"])
    qlw, qlb = f(inputs["qlw"]), f(inputs["qlb"])
    klw, klb = f(inputs["klw"]), f(inputs["klb"])
    alw, alb = f(inputs["alw"]), f(inputs["alb"])
    l1w, l1b = f(inputs["l1w"]), f(inputs["l1b"])
    for g, bb in [("qng", "qnb"), ("kng", "knb"), ("vng", "vnb")]:
        assert np.all(inputs[g] == 1.0) and np.all(inputs[bb] == 0.0), (
            "non-identity LayerNorm affine not supported by this kernel"
        )

    mm = lambda a: np.ascontiguousarray(a.astype(mmdt_np))

    # xt: [B, 27, N] = x channels + coords + ones row
    xt = np.empty((B, F1, N), np.float32)
    xt[:, :F, :] = x.reshape(B, F, N)
    xt[:, F, :] = np.tile(np.arange(IW, dtype=np.float32) / IW, IH)
    xt[:, F + 1, :] = np.repeat(np.arange(IH, dtype=np.float32) / IH, IW)
    xt[:, F + 2, :] = 1.0

    # head-interleaved Q|K projection weights (bias in last row)
    qp = np.concatenate([qpw, qpb[:, None]], 1).T   # [27, 256]
    kp = np.concatenate([kpw, kpb[:, None]], 1).T
    wqk = np.empty((F1, 512), np.float32)
    for h in range(HEADS):
        wqk[:, h * 128:h * 128 + 64] = qp[:, h * 64:(h + 1) * 64]
        wqk[:, h * 128 + 64:h * 128 + 128] = kp[:, h * 64:(h + 1) * 64]
    wv = np.concatenate([vpw, vpb[:, None]], 1).T   # [27, 256]

    wcat = np.concatenate([qlw.T, klw.T], 0)        # [128, N]

    # alw transposed, pre-scaled, quantized to e4m3 for the DoubleRow matmul
    alwt_q8 = (alw.T * ALW_SCALE).astype(ml_dtypes.float8_e4m3)
    alwt_deq = alwt_q8.astype(np.float32) / ALW_SCALE        # [m, p]
    alwt8 = np.ascontiguousarray(alwt_q8.reshape(NT, 128, N))

    l1wt = l1w.T.reshape(2, 128, D)

    bvec = qlb + klb                                # posb, [m]
    # device stores st = elu(u) + 1 - posb; fold the +1 and -posb through
    # the QUANTIZED alw so they cancel exactly:
    albe = alb - (alwt_deq * (1.0 - bvec)[:, None]).sum(axis=0)
    posb = bvec.reshape(NT, 128).T.copy()           # [128, NT]
    albe = albe.reshape(NT, 128).T.copy()

    ar = np.arange(128)
    ind = np.stack([(ar < 64).astype(np.float32),
                    (ar >= 64).astype(np.float32)], 1)

    smalls = np.zeros((128, 3 * NT + 3), np.float32)
    smalls[:, 0:NT] = posb
    smalls[:, NT:2 * NT] = -posb
    smalls[:, 2 * NT:3 * NT] = albe
    smalls[:, 3 * NT:3 * NT + 2] = ind
    smalls[0:D, 3 * NT + 2] = l1b
    shared = {
        "wqk": mm(wqk), "wv": mm(wv),
        "wcat": mm(wcat), "alwt": alwt8, "l1wt": mm(l1wt),
        "smalls": smalls,
    }
    in_maps = []
    for c in range(NCORES):
        m = dict(shared)
        m["xt"] = np.ascontiguousarray(xt[c * BL:(c + 1) * BL].astype(mmdt_np))
        in_maps.append(m)
    return in_maps


_CACHE = {}


def _get_program(mmdt, debug):
    key = (str(mmdt), debug)
    if key not in _CACHE:
        _CACHE[key] = _build(mmdt, debug)
    return _CACHE[key]


def run(inputs, mmdt="f16", debug=False, trace=False):
    dt = {"bf16": mybir.dt.bfloat16, "f16": mybir.dt.float16, "f32": FP32}[mmdt]
    dt_np = {"bf16": ml_dtypes.bfloat16, "f16": np.float16, "f32": np.float32}[mmdt]
    nc = _get_program(dt, debug)
    in_maps = _prep_inputs(inputs, dt_np)
    res = bass_utils.run_bass_kernel_spmd(
        nc, in_maps, core_ids=list(range(NCORES)), trace=trace
    )
    out = np.concatenate([r["out"] for r in res.results], 0).astype(np.float32)
    return out, res


def kernel(**inputs):
    out, _ = run(inputs, mmdt=os.environ.get("MHR_MMDT", "bf16"))
    return out


# revision 22
# speedup vs baseline: 1.0020x; 1.0020x over previous
"""Trainium2 Bass kernel: multi-head relational module (dense_transformer).

Computation (per batch b):
  xin = concat(x, xy-coords)                 [N=1024, FEAT=26]
  Q/K/V = LN_global(xin @ Wp.T + bp)         LN over all (heads, N, D) per b
  S1 = elu(Q @ qlw.T + qlb + K @ klw.T + klb)      [h, N, N]
  A  = softmax(S1 @ alw.T + alb, axis=-1)          [h, N, N]
  E  = relu((A @ V).reshape(N, 256) @ l1w.T + l1b) [N, 64]
  out[b] = max_n LN(E)                             [64]

Sharding: data-parallel over batch (B=16 -> 2 per core on 8 cores).

Precision plan:
  * Projections, S1 additive-attention matmuls: fp16/bf16 (fp32 PSUM accum).
  * The two big contractions (S1 @ alw.T at 84% of FLOPs, and A @ V) run in
    fp8e4 (e4m3) with perf_mode=DoubleRow: contraction chunks are paired
    [128, 2, free] so one matmul instruction contracts 256 rows (~2x
    TensorE throughput, measured 216 ns per 128x256x512 matmul). alw is
    pre-scaled by ALW_SCALE on the host to move its mass out of the e4m3
    subnormal range; the softmax exp un-scales via its `scale` input.

Engine balance choices (from perfetto/ntff analysis):
  * S1 elu chain per 128x1024 chunk is ONE 2-bank PSUM tile + 3 ops:
      ep = exp(pss + posb)              [ACT]
      qq = min(ep, 1)                   [DVE 4x tensor_scalar, immediates]
      st = (pss max -posb) + qq         [DVE, the only PSUM read]
    st == elu(u)+1-posb; the -posb and the "+1" are folded into albe on
    the host (albe = alb - sum_m alw_q[:,m] * (1 - posb[m])).
  * All LayerNorm partition-reductions and the softmax-denominator
    broadcast run on the otherwise-idle GpSimd engine
    (partition_all_reduce / partition_broadcast), which frees the PE from
    ~60 small matmuls and frees a PSUM bank (no pmisc pool).
  * LayerNorm rstd is computed as exp(-0.5*ln(v+eps)) so the ACT engine
    stays in the single `natural_log_exp_and_others` table set.
"""

import os
from contextlib import ExitStack

import ml_dtypes
import numpy as np

import concourse.bass as bass
import concourse.bass_utils as bass_utils
import concourse.mybir as mybir
import concourse.tile as tile
from concourse import bacc
from concourse.bass_isa import ReduceOp

AF = mybir.ActivationFunctionType
ALU = mybir.AluOpType
FP32 = mybir.dt.float32
F8 = mybir.dt.float8e4
DR = mybir.MatmulPerfMode.DoubleRow

B, F, IH, IW = 16, 24, 32, 32
N = IH * IW          # 1024
HEADS, D = 4, 64
HD = HEADS * D       # 256
FEAT = F + 2         # 26
F1 = FEAT + 1        # 27 (ones row folds the projection bias in)
EPS = 1e-5
NCORES = 8
BL = B // NCORES     # batches per core
NT = N // 128        # 8 chunks of the node axis
NJ = NT // 2         # 4 DoubleRow chunk-pairs
NPAIRS = BL * HEADS  # 8 (b, h) pairs per core
ALW_SCALE = 16.0     # host-side pre-scale of alw before e4m3 quantization


def _build(mmdt, debug=False):
    """Build + compile the per-core Bass program. Same program on all cores."""
    nc = bacc.Bacc("TRN2", target_bir_lowering=False, debug=False)

    def din(name, shape, dt=FP32):
        return nc.dram_tensor(name, shape, dt, kind="ExternalInput").ap()

    xt_d = din("xt", [BL, F1, N], mmdt)
    wqk_d = din("wqk", [F1, 512], mmdt)
    wv_d = din("wv", [F1, HD], mmdt)
    wcat_d = din("wcat", [128, N], mmdt)
    alwt_d = din("alwt", [NT, 128, N], F8)
    l1wt_d = din("l1wt", [2, 128, D], mmdt)
    smalls_d = din("smalls", [128, 3 * NT + 3])
    out_d = nc.dram_tensor("out", [BL, D], FP32, kind="ExternalOutput").ap()

    dbg = {}
    if debug:
        for nm, shape, dt in [
            ("dbg_qkcat", [NPAIRS, 128, N], mmdt),
            ("dbg_vt", [BL * NJ, 128, 2, HEADS, 128], F8),
            ("dbg_s1t", [NJ, 128, 2, N], F8),
            ("dbg_expt", [NJ, 128, 2, N], F8),
            ("dbg_eflat", [BL, 2, 128, N], mmdt),
            ("dbg_orelu", [BL, 64, N], FP32),
        ]:
            dbg[nm] = nc.dram_tensor(nm, shape, dt, kind="ExternalOutput").ap()

    with tile.TileContext(nc) as tc, ExitStack() as ctx:
        pool = lambda name, bufs, **kw: ctx.enter_context(
            tc.tile_pool(name=name, bufs=bufs, **kw)
        )
        # PSUM: 8 banks total, budgeted exactly:
        #   pa2: 2 bufs x 1 bank (A2 accum; also QK/V proj + out-lin psums)
        #   ps1: 2 bufs x 2 banks (merged-half S1 psum [128,1024])
        #   pe:  2 bufs x 1 bank (E accumulation halves)
        pa2 = pool("pa2", 2, space="PSUM")
        ps1 = pool("ps1", 2, space="PSUM")
        pe_ = pool("pe", 2, space="PSUM")

        consts = pool("consts", 1)
        sxt = pool("sxt", 2)
        sqkraw = pool("sqkraw", 9)
        svraw = pool("svraw", 17)
        sqkcat = pool("sqkcat", NPAIRS)
        svt = pool("svt", BL * NJ)
        ss1t = pool("ss1t", 14)
        sexpt = pool("sexpt", 8)
        selu = pool("selu", 8)
        seflat = pool("seflat", 4)
        sorelu = pool("sorelu", 2)
        ssmall = pool("ssmall", 4)
        ssb = pool("ssb", 6)

        # ---- constants into SBUF (small/urgent first; big weights last) ----
        wqk_s = consts.tile([F1, 512], mmdt)
        nc.sync.dma_start(out=wqk_s, in_=wqk_d)
        wv_s = consts.tile([F1, HD], mmdt)
        nc.sync.dma_start(out=wv_s, in_=wv_d)
        xt_tiles = []
        for b in range(BL):
            xt_s = sxt.tile([F1, N], mmdt, name=f"xt{b}", tag="xt")
            nc.scalar.dma_start(out=xt_s, in_=xt_d[b])
            xt_tiles.append(xt_s)
        smalls_s = consts.tile([128, 3 * NT + 3], FP32)
        nc.sync.dma_start(out=smalls_s, in_=smalls_d)
        posb_s = smalls_s[:, 0:NT]
        nposb_s = smalls_s[:, NT:2 * NT]
        albe_s = smalls_s[:, 2 * NT:3 * NT]
        ind_s = smalls_s[:, 3 * NT:3 * NT + 2]
        l1b_s = smalls_s[0:D, 3 * NT + 2:3 * NT + 3]
        eps_s = consts.tile([128, 1], FP32)
        nc.vector.memset(eps_s, EPS)
        wcat_s = consts.tile([128, N], mmdt)
        nc.sync.dma_start(out=wcat_s, in_=wcat_d)
        alwt_s = consts.tile([128, NT, N], F8)
        for mi in range(NT):
            nc.sync.dma_start(out=alwt_s[:, mi, :], in_=alwt_d[mi])
        l1wt_s = consts.tile([128, 2, D], mmdt)
        for c in range(2):
            nc.sync.dma_start(out=l1wt_s[:, c, :], in_=l1wt_d[c])

        qkcat = {}   # (b, h) -> [128, N] mmdt   rows: 0-63 Q_h.T dims, 64-127 K_h.T
        vt = {}      # (b, j) -> [128, 2, HEADS, 80] F8 (col 64 per head = ones)
        eflat = {}   # (b, c) -> [128, N] mmdt   E.T stacked by head pairs

        def ln_scalars(stats, parts, cnt, name):
            """From SBUF stats [parts,3] = (sum mean_p, sum var_p, sum mean_p^2)
            build sbuf [parts,2] = (rstd, -mean*rstd). All per-partition."""
            stm = ssmall.tile([parts, 8], FP32, name=f"stm_{name}", tag="stm")
            nc.scalar.activation(stm[:, 0:3], stats, AF.Copy, scale=1.0 / cnt)
            nc.vector.tensor_mul(stm[:, 3:4], stm[:, 0:1], stm[:, 0:1])
            nc.vector.tensor_add(stm[:, 4:5], stm[:, 1:2], stm[:, 2:3])
            nc.vector.tensor_sub(stm[:, 5:6], stm[:, 4:5], stm[:, 3:4])
            # rstd = exp(-0.5 * ln(var + eps)); keeps ACT in one table set
            nc.scalar.activation(stm[:, 6:7], stm[:, 5:6], AF.Ln, bias=eps_s[0:parts])
            rhs2 = ssb.tile([parts, 2], FP32, name=f"rhs2_{name}", tag="sbb")
            nc.scalar.activation(rhs2[:, 0:1], stm[:, 6:7], AF.Exp, scale=-0.5)
            nc.vector.tensor_mul(stm[:, 7:8], stm[:, 0:1], rhs2[:, 0:1])
            nc.vector.tensor_scalar(
                rhs2[:, 1:2], stm[:, 7:8], -1.0, None, ALU.mult
            )
            return rhs2

        def agg_stats(st6, parts, name):
            """bn_aggr the [parts, nsub, 6] bn_stats + mean^2 -> [parts, 3]."""
            rhs3 = ssmall.tile([parts, 3], FP32, name=f"rhs3_{name}", tag="rhs3")
            nc.vector.bn_aggr(rhs3[:, 0:2], st6)
            nc.vector.tensor_mul(rhs3[:, 2:3], rhs3[:, 0:1], rhs3[:, 0:1])
            return rhs3

        # ================= stage 0: projections + LayerNorms =================
        def emit_stage0_qk(b):
            xt_s = xt_tiles[b]

            # --- combined Q|K projection, head-interleaved, transposed ---
            # batch 0 runs before the pair loop: borrow the idle ps1 pool so
            # four proj psums are in flight instead of two
            qkraw = []
            qst6 = ssmall.tile([128, 2 * HEADS, 6], FP32,
                               name=f"qst6_{b}", tag="st6")
            for h in range(HEADS):
                pools = {0: (pa2, "pa2"), 1: (ps1, "ps1"),
                         2: (pe_, "pe"), 3: (pa2, "pa2")}
                pp, tg = pools[h]
                ps = pp.tile([128, 512], FP32, name=f"pqk{b}{h}", tag=tg)
                ps2 = pp.tile([128, 512], FP32, name=f"pqk2{b}{h}", tag=tg)
                lhs = wqk_s[:, h * 128:(h + 1) * 128]
                nc.tensor.matmul(ps, lhs, xt_s[:, 0:512])
                nc.tensor.matmul(ps2, lhs, xt_s[:, 512:1024])
                raw = sqkraw.tile([128, N], FP32, name=f"qkraw{b}{h}", tag="qkraw")
                nc.scalar.activation(raw[:, 0:512], ps, AF.Copy)
                nc.scalar.activation(raw[:, 512:1024], ps2, AF.Copy)
                qkraw.append(raw)
                hv = raw.rearrange("p (s f) -> p s f", s=2)
                nc.vector.bn_stats(qst6[:, 2 * h, :], hv[:, 0, :])
                nc.vector.bn_stats(qst6[:, 2 * h + 1, :], hv[:, 1, :])
            rhs3 = agg_stats(qst6, 128, f"qk{b}")
            # rows 0-63 hold Q-dim stats, 64-127 K-dim stats: group-reduce via
            # masked columns + one full-partition all-reduce, then select.
            acc6 = ssmall.tile([128, 6], FP32, name=f"acc6_{b}", tag="acc6")
            nc.vector.memset(acc6, 0.0)
            nc.vector.tensor_copy(acc6[0:64, 0:3], rhs3[0:64, :])
            nc.vector.tensor_copy(acc6[64:128, 3:6], rhs3[64:128, :])
            nc.gpsimd.partition_all_reduce(acc6, acc6, 128, ReduceOp.add)
            sel = ssmall.tile([128, 3], FP32, name=f"sel_{b}", tag="rhs3")
            nc.vector.tensor_scalar(
                sel, acc6[:, 0:3], ind_s[:, 0:1], None, ALU.mult
            )
            nc.vector.scalar_tensor_tensor(
                sel, acc6[:, 3:6], ind_s[:, 1:2], sel, ALU.mult, ALU.add
            )
            sbb = ln_scalars(sel, 128, 64.0, f"qk{b}")
            for h in range(HEADS):
                qk = sqkcat.tile([128, N], mmdt, name=f"qkcat{b}{h}", tag="qkcat")
                nc.vector.tensor_scalar(
                    qk, qkraw[h], sbb[:, 0:1], sbb[:, 1:2], ALU.mult, ALU.add
                )
                qkcat[(b, h)] = qk

        def emit_stage0_v(b):
            xt_s = xt_tiles[b]
            # --- V projection, natural [n, hd] layout ---
            vraws = []
            vst6 = ssmall.tile([128, NT, 6], FP32, name=f"vst6_{b}", tag="st6")
            for nt in range(NT):
                pp = ps1 if nt % 2 == 1 else pa2
                tg = "ps1" if nt % 2 == 1 else "pa2"
                psv = pp.tile([128, HD], FP32, name=f"pv{b}{nt}", tag=tg)
                nc.tensor.matmul(
                    psv, xt_s[:, nt * 128:(nt + 1) * 128], wv_s
                )
                vr = svraw.tile([128, HD], FP32, name=f"vraw{b}{nt}", tag="vraw")
                nc.scalar.activation(vr, psv, AF.Copy)
                vraws.append(vr)
                nc.vector.bn_stats(vst6[:, nt, :], vr)
            rhs3v = agg_stats(vst6, 128, f"v{b}")
            nc.gpsimd.partition_all_reduce(rhs3v, rhs3v, 128, ReduceOp.add)
            sbv = ln_scalars(rhs3v, 128, 128.0, f"v{b}")
            for nt in range(NT):
                j, c = divmod(nt, 2)
                if c == 0:
                    v = svt.tile([128, 2, HEADS, 128], F8,
                                 name=f"vt{b}{j}", tag="vt")
                    vt[(b, j)] = v
                v = vt[(b, j)]
                nc.vector.tensor_scalar(
                    v[:, c, :, 64:128],
                    vraws[nt].rearrange("p (h d) -> p h d", h=HEADS),
                    sbv[:, 0:1], sbv[:, 1:2], ALU.mult, ALU.add,
                )
                nc.vector.memset(v[:, c, :, 0:1], 1.0)
                nc.vector.memset(v[:, c, :, 1:64], 0.0)

        # ================= stage 1: attention pipeline over (b, h) ==========
        s1p = {}    # (i, j) -> [128, 2, N] F8    S1.T chunk pair (elu+1-posb)
        exp2 = {}   # (i, j) -> [128, 2, N] F8    exp(A2T) chunk pair
        eps_ = {}   # (i, half) -> psum [65, 512] E.T accumulation + denominator

        def emit_s1_chunk(i, mi):
            b, h = divmod(i, HEADS)
            j, c = divmod(mi, 2)
            if (i, j) not in s1p:
                s1p[(i, j)] = ss1t.tile(
                    [128, 2, N], F8, name=f"s1p{i}_{j}", tag="s1t"
                )
            sp = s1p[(i, j)]
            pss = ps1.tile([128, N], FP32, name=f"ps1_{i}_{mi}", tag="ps1")
            wc = wcat_s[:, mi * 128:(mi + 1) * 128]
            nc.tensor.matmul(pss[:, 0:512], wc, qkcat[(b, h)][:, 0:512])
            nc.tensor.matmul(pss[:, 512:1024], wc, qkcat[(b, h)][:, 512:1024])
            # st = relu(u) + min(exp(u), 1) - posb, u = pss + posb:
            #   ep = exp(u)                 [ACT, PSUM->SBUF]
            #   qq = min(ep, 1)             [DVE 4x tensor_scalar, immediate]
            #   st = (pss max -posb) + qq   [DVE, the only PSUM read]
            ep = selu.tile([128, N], mmdt, name=f"ep{i}{mi}", tag="ep")
            nc.scalar.activation(ep, pss, AF.Exp, bias=posb_s[:, mi:mi + 1])
            qq = selu.tile([128, N], mmdt, name=f"qq{i}{mi}", tag="qq")
            nc.vector.tensor_scalar(qq, ep, 1.0, None, ALU.min)
            nc.vector.scalar_tensor_tensor(
                sp[:, c, :], pss, nposb_s[:, mi:mi + 1], qq, ALU.max, ALU.add
            )

        def emit_a2_ptile(i, pt):
            j2, c2 = divmod(pt, 2)
            if (i, j2) not in exp2:
                exp2[(i, j2)] = sexpt.tile(
                    [128, 2, N], F8, name=f"expt{i}_{j2}", tag="expt"
                )
            ex = exp2[(i, j2)]
            for half in range(2):
                sl = slice(half * 512, half * 512 + 512)
                psa = pa2.tile([128, 512], FP32,
                               name=f"pa2_{i}_{pt}{half}", tag="pa2")
                for j in range(NJ):
                    nc.tensor.matmul(
                        psa,
                        alwt_s[:, 2 * j:2 * j + 2, pt * 128:(pt + 1) * 128],
                        s1p[(i, j)][:, :, sl],
                        start=(j == 0), stop=(j == NJ - 1),
                        perf_mode=DR,
                    )
                nc.scalar.activation(
                    ex[:, c2, sl], psa, AF.Exp,
                    bias=albe_s[:, pt:pt + 1], scale=1.0 / ALW_SCALE,
                )

        def emit_e_mms(i, j):
            b, h = divmod(i, HEADS)
            for half in range(2):
                if j == 0:
                    eps_[(i, half)] = pe_.tile(
                        [128, 512], FP32, name=f"pe{i}{half}", tag="pe"
                    )
                sl = slice(half * 512, half * 512 + 512)
                nc.tensor.matmul(
                    eps_[(i, half)], vt[(b, j)][:, :, h, 0:128],
                    exp2[(i, j)][:, :, sl],
                    start=(j == 0), stop=(j == NJ - 1),
                    perf_mode=DR,
                )

        def emit_div(i):
            b, h = divmod(i, HEADS)
            c, po = h // 2, (h % 2) * 64
            if (b, c) not in eflat:
                eflat[(b, c)] = seflat.tile(
                    [128, N], mmdt, name=f"eflat{b}{c}", tag="eflat"
                )
            for half in range(2):
                sl = slice(half * 512, half * 512 + 512)
                pE = eps_[(i, half)]
                # ones column is FIRST in vt (V dims at 32:96 for the 32-aligned
                # partition-base rule), so the denominator is PSUM row 0
                # (a zero-base-partition read, which DVE handles; other
                # PSUM base partitions silently read partition 0 on HW).
                rc = ssmall.tile([1, 512], FP32, name=f"rc{i}{half}", tag="rc")
                nc.vector.reciprocal_approx_fast(out=rc, in_=pE[0:1, :])
                bc = ssb.tile([64, 512], FP32, name=f"bc{i}{half}", tag="bc")
                nc.gpsimd.partition_broadcast(bc, rc, channels=64)
                nc.vector.tensor_mul(
                    eflat[(b, c)][po:po + 64, sl], pE[64:128, :], bc
                )

        def emit_outlin(b):
            orl = sorelu.tile([64, N], FP32, name=f"orelu{b}", tag="orelu")
            ost6 = ssmall.tile([64, 2, 6], FP32, name=f"ost6_{b}", tag="ost6")
            rmax2 = ssmall.tile([64, 2], FP32, name=f"rmax2_{b}", tag="rmax2")
            for half in range(2):
                sl = slice(half * 512, half * 512 + 512)
                pso = pa2.tile([64, 512], FP32,
                               name=f"po{b}{half}", tag="pa2")
                for c in range(2):
                    nc.tensor.matmul(
                        pso, l1wt_s[:, c, :], eflat[(b, c)][:, sl],
                        start=(c == 0), stop=(c == 1),
                    )
                nc.scalar.activation(orl[:, sl], pso, AF.Relu, bias=l1b_s)
                nc.vector.bn_stats(ost6[:, half, :], orl[:, sl])
                nc.vector.reduce_max(rmax2[:, half:half + 1], orl[:, sl],
                                     axis=mybir.AxisListType.X)
            rhs3o = agg_stats(ost6, 64, f"o{b}")
            nc.gpsimd.partition_all_reduce(rhs3o, rhs3o, 64, ReduceOp.add)
            sbo = ln_scalars(rhs3o, 64, 64.0, f"o{b}")
            rmax = ssmall.tile([64, 1], FP32, name=f"rmax{b}", tag="rmax")
            nc.vector.tensor_max(rmax, rmax2[:, 0:1], rmax2[:, 1:2])
            ob = ssmall.tile([64, 1], FP32, name=f"ob{b}", tag="ob")
            nc.vector.tensor_scalar(
                ob, rmax, sbo[:, 0:1], sbo[:, 1:2], ALU.mult, ALU.add
            )
            nc.sync.dma_start(
                out=out_d[b:b + 1, :].rearrange("o d -> d o"), in_=ob
            )
            if debug:
                nc.sync.dma_start(out=dbg["dbg_orelu"][b], in_=orl)

        emit_stage0_qk(0)
        emit_stage0_v(0)
        emit_stage0_qk(1)
        emit_stage0_v(1)
        for mi in range(NT):
            emit_s1_chunk(0, mi)
        for i in range(NPAIRS):
            b, h = divmod(i, HEADS)
            for pt in range(NT):
                # prefetch next pair's S1 chunks one p-tile ahead so the elu
                # chain finishes before A2(i+1) starts consuming them
                if i + 1 < NPAIRS:
                    if pt == 0:
                        emit_s1_chunk(i + 1, 0)
                    if pt < NT - 1:
                        emit_s1_chunk(i + 1, pt + 1)
                if pt == 0 and i > 0:
                    # deferred: previous pair's softmax division (its recip
                    # chain latency hides behind A2(i, 0..4))
                    emit_div(i - 1)
                if pt == 5 and i > 0 and h == 0:
                    # five p-tiles later so the div chain (DVE recip -> gpsimd
                    # broadcast -> DVE mul) is done before the PE needs eflat
                    emit_outlin(b - 1)
                if pt >= 2 and pt % 2 == 0:
                    emit_e_mms(i, pt // 2 - 1)
                emit_a2_ptile(i, pt)
            emit_e_mms(i, NJ - 1)
            if debug and i == 0:
                for k in range(NJ):
                    nc.sync.dma_start(out=dbg["dbg_s1t"][k], in_=s1p[(0, k)])
                    nc.sync.dma_start(out=dbg["dbg_expt"][k], in_=exp2[(0, k)])
        emit_div(NPAIRS - 1)
        emit_outlin(BL - 1)

        if debug:
            for (b, h), t in qkcat.items():
                nc.sync.dma_start(out=dbg["dbg_qkcat"][b * HEADS + h], in_=t)
            for (b, j), t in vt.items():
                nc.sync.dma_start(out=dbg["dbg_vt"][b * NJ + j], in_=t)
            for (b, c), t in eflat.items():
                nc.sync.dma_start(out=dbg["dbg_eflat"][b, c], in_=t)

    import concourse.bacc as bacc_mod
    from concourse.hw_specs import get_activation_tables

    full = get_activation_tables(nc.m.arch)
    mine = full["natural_log_exp_and_others"]
    # Keep dict order/length (act_func_set_id indexes the full list); make
    # every other set unable to serve our functions so one set is loaded once.
    pinned = {
        name: (fns if name == "natural_log_exp_and_others" else fns - mine)
        for name, fns in full.items()
    }
    orig_gat = bacc_mod.get_activation_tables
    bacc_mod.get_activation_tables = lambda arch: pinned
    try:
        nc.compile()
    finally:
        bacc_mod.get_activation_tables = orig_gat
    return nc


def _prep_inputs(inputs, mmdt_np):
    """Host-side: shard + lay out all tensors exactly as SBUF wants them."""
    f = lambda a: np.ascontiguousarray(np.asarray(a, np.float32))
    x = f(inputs["x"])
    qpw, qpb = f(inputs["qpw"]), f(inputs["qpb"])
    kpw, kpb = f(inputs["kpw"]), f(inputs["kpb"])
    vpw, vpb = f(inputs["vpw"]), f(inputs["vpb"])
    qlw, qlb = f(inputs["qlw"]), f(inputs["qlb"])
    klw, klb = f(inputs["klw"]), f(inputs["klb"])
    alw, alb = f(inputs["alw"]), f(inputs["alb"])
    l1w, l1b = f(inputs["l1w"]), f(inputs["l1b"])
    for g, bb in [("qng", "qnb"), ("kng", "knb"), ("vng", "vnb")]:
        assert np.all(inputs[g] == 1.0) and np.all(inputs[bb] == 0.0), (
            "non-identity LayerNorm affine not supported by this kernel"
        )

    mm = lambda a: np.ascontiguousarray(a.astype(mmdt_np))

    # xt: [B, 27, N] = x channels + coords + ones row
    xt = np.empty((B, F1, N), np.float32)
    xt[:, :F, :] = x.reshape(B, F, N)
    xt[:, F, :] = np.tile(np.arange(IW, dtype=np.float32) / IW, IH)
    xt[:, F + 1, :] = np.repeat(np.arange(IH, dtype=np.float32) / IH, IW)
    xt[:, F + 2, :] = 1.0

    # head-interleaved Q|K projection weights (bias in last row)
    qp = np.concatenate([qpw, qpb[:, None]], 1).T   # [27, 256]
    kp = np.concatenate([kpw, kpb[:, None]], 1).T
    wqk = np.empty((F1, 512), np.float32)
    for h in range(HEADS):
        wqk[:, h * 128:h * 128 + 64] = qp[:, h * 64:(h + 1) * 64]
        wqk[:, h * 128 + 64:h * 128 + 128] = kp[:, h * 64:(h + 1) * 64]
    wv = np.concatenate([vpw, vpb[:, None]], 1).T   # [27, 256]

    wcat = np.concatenate([qlw.T, klw.T], 0)        # [128, N]

    # alw transposed, pre-scaled, quantized to e4m3 for the DoubleRow matmul
    alwt_q8 = (alw.T * ALW_SCALE).astype(ml_dtypes.float8_e4m3)
    alwt_deq = alwt_q8.astype(np.float32) / ALW_SCALE        # [m, p]
    alwt8 = np.ascontiguousarray(alwt_q8.reshape(NT, 128, N))

    l1wt = l1w.T.reshape(2, 128, D)

    bvec = qlb + klb                                # posb, [m]
    # device stores st = elu(u) + 1 - posb; fold the +1 and -posb through
    # the QUANTIZED alw so they cancel exactly:
    albe = alb - (alwt_deq * (1.0 - bvec)[:, None]).sum(axis=0)
    posb = bvec.reshape(NT, 128).T.copy()           # [128, NT]
    albe = albe.reshape(NT, 128).T.copy()

    ar = np.arange(128)
    ind = np.stack([(ar < 64).astype(np.float32),
                    (ar >= 64).astype(np.float32)], 1)

    smalls = np.zeros((128, 3 * NT + 3), np.float32)
    smalls[:, 0:NT] = posb
    smalls[:, NT:2 * NT] = -posb
    smalls[:, 2 * NT:3 * NT] = albe
    smalls[:, 3 * NT:3 * NT + 2] = ind
    smalls[0:D, 3 * NT + 2] = l1b
    shared = {
        "wqk": mm(wqk), "wv": mm(wv),
        "wcat": mm(wcat), "alwt": alwt8, "l1wt": mm(l1wt),
        "smalls": smalls,
    }
    in_maps = []
    for c in range(NCORES):
        m = dict(shared)
        m["xt"] = np.ascontiguousarray(xt[c * BL:(c + 1) * BL].astype(mmdt_np))
        in_maps.append(m)
    return in_maps


_CACHE = {}


def _get_program(mmdt, debug):
    key = (str(mmdt), debug)
    if key not in _CACHE:
        _CACHE[key] = _build(mmdt, debug)
    return _CACHE[key]


def run(inputs, mmdt="f16", debug=False, trace=False):
    dt = {"bf16": mybir.dt.bfloat16, "f16": mybir.dt.float16, "f32": FP32}[mmdt]
    dt_np = {"bf16": ml_dtypes.bfloat16, "f16": np.float16, "f32": np.float32}[mmdt]
    nc = _get_program(dt, debug)
    in_maps = _prep_inputs(inputs, dt_np)
    res = bass_utils.run_bass_kernel_spmd(
        nc, in_maps, core_ids=list(range(NCORES)), trace=trace
    )
    out = np.concatenate([r["out"] for r in res.results], 0).astype(np.float32)
    return out, res


def kernel(**inputs):
    out, _ = run(inputs, mmdt=os.environ.get("MHR_MMDT", "bf16"))
    return out
